# revision 6
# baseline (speedup 1.0000x reference)
"""GNN message-passing + MLP/BN kernel for 8 trn2 NeuronCores.

Math (algebraically identical to the reference; the degree terms cancel):
    h[v]  = (1+eps)*x[v] + sum_{edges e incident to v} (x[other(e)] + ea[e])
    z1    = h @ W1.T ; y1 = relu(bn1(z1))   (BatchNorm over the full batch)
    z2    = y1 @ W2.T; out = relu(bn2(z2))

Distribution: nodes are block-partitioned across 8 cores (12500 each).
The host builds, per core, an incidence list sorted by (node tile of 128,
bank of the gathered endpoint); edge features are packed per core in
incidence order (graph partitioning / halo duplication), node features are
replicated per core and gathered on-device with dma_gather. Scatter-add
into nodes is a one-hot matmul accumulated in PSUM. BatchNorm statistics
are all-reduced across the 8 cores on-device.
"""
import os
import sys
import numpy as np

for _p in ("/opt/trn_rl_repo", "/root/.axon_site/_ro/trn_rl_repo"):
    if os.path.isdir(_p) and _p not in sys.path:
        sys.path.insert(0, _p)

import concourse.bacc as bacc
import concourse.bass as bass
import concourse.mybir as mybir
import concourse.tile as tile
from concourse import bass_utils

NCORES = 8
BANK = 25000          # dma_gather int16 index range per bank
BN_EPS = 1e-5
P = 128

F32 = mybir.dt.float32
I16 = mybir.dt.int16


# ----------------------------------------------------------------------------
# Host-side graph structure
# ----------------------------------------------------------------------------

def _build_structure(src, dst, n):
    """Sorted incidence structure shared by all cores.

    Returns dict with per-(core,tile,bank) slot assignment of the 2E
    incidences (owner node, other endpoint, edge id)."""
    e = src.shape[0]
    npc = n // NCORES
    nt = (npc + P - 1) // P
    nb = (n + BANK - 1) // BANK

    owner = np.concatenate([src, dst])
    other = np.concatenate([dst, src])
    eid = np.concatenate([np.arange(e, dtype=np.int64)] * 2)

    core_k = owner // npc
    tile_k = (owner % npc) // P
    bank_k = other // BANK
    key = (core_k * nt + tile_k) * nb + bank_k
    order = np.argsort(key, kind="stable")
    owner_s = owner[order]
    other_s = other[order]
    eid_s = eid[order]
    key_s = key[order]

    counts = np.bincount(key_s, minlength=NCORES * nt * nb).reshape(NCORES, nt, nb)
    kb = -(-counts // P)            # ceil chunks per (core,tile,bank)
    kb = kb.max(axis=0)             # shared across cores [nt, nb]
    kb[:, 0] = np.maximum(kb[:, 0], 1)   # every tile gets >=1 chunk
    k_t = kb.sum(axis=1)            # chunks per tile [nt]
    koff = np.concatenate([[0], np.cumsum(k_t)])[:-1]          # chunk offset per tile
    kboff = np.concatenate([np.zeros((nt, 1), np.int64),
                            np.cumsum(kb, axis=1)], axis=1)[:, :-1]  # within tile
    totk = int(k_t.sum())

    # group chunk base for (t,b): gc0 = koff[t] + kboff[t,b]
    gc0 = koff[:, None] + kboff                                 # [nt, nb]

    # per-incidence destination slot: gc0[t,b]*128 + rank within (c,t,b) run
    run_bounds = np.concatenate([[0], np.cumsum(counts.reshape(-1))])
    rank = np.arange(owner.shape[0], dtype=np.int64) - run_bounds[key_s]
    t_of = (owner_s % npc) // P
    b_of = other_s // BANK
    dest = gc0[t_of, b_of] * P + rank                           # slot within core

    return dict(npc=npc, nt=nt, nb=nb, kb=kb, k_t=k_t, koff=koff, gc0=gc0,
                totk=totk, owner_s=owner_s, other_s=other_s, eid_s=eid_s,
                core_of=core_k[order], dest=dest)


def _pack_core(st, c, node_rep, edge_attr, msg_np):
    """Per-core input arrays."""
    npc, nt, totk = st["npc"], st["nt"], st["totk"]
    npad = nt * P
    sel = st["core_of"] == c
    owner = st["owner_s"][sel]
    other = st["other_s"][sel]
    eid = st["eid_s"][sel]
    dest = st["dest"][sel]

    base = c * npc
    # int16 gather indices, layout [16, totk*8] (idx j at (j%16, j//16)), x8 rows
    a16 = np.zeros((16, totk * 8), np.int16)
    a16[dest % 16, dest // 16] = (other % BANK).astype(np.int16)
    gx = np.tile(a16, (8, 1))

    lid = np.full((P, totk), -1.0, np.float32)
    lid[dest % P, dest // P] = (owner - base) % P

    eidx = np.zeros((P, totk), np.int64)
    eidx[dest % P, dest // P] = eid
    eap = edge_attr[eidx.reshape(-1)].astype(msg_np).reshape(P, totk * 128)

    xt = np.zeros((P, npad), np.float32)
    xt[:, :npc] = node_rep[base:base + npc].T
    return dict(gx=gx, lid=lid, eap=eap, xt=xt)


# ----------------------------------------------------------------------------
# Device program
# ----------------------------------------------------------------------------

def _build_nc(st, n, msg_dt):
    npc, nt, nb, totk = st["npc"], st["nt"], st["nb"], st["totk"]
    kb, k_t, koff, gc0 = st["kb"], st["k_t"], st["koff"], st["gc0"]
    npad = nt * P
    kmax = int(k_t.max())
    H = 256
    D = 128

    nc = bacc.Bacc("TRN2", target_bir_lowering=False, debug=False,
                   num_devices=NCORES)
    x_in = nc.dram_tensor("x", [n, D], msg_dt, kind="ExternalInput")
    gx_in = nc.dram_tensor("gx", [P, totk * 8], I16, kind="ExternalInput")
    lid_in = nc.dram_tensor("lid", [P, totk], F32, kind="ExternalInput")
    ea_in = nc.dram_tensor("eap", [P, totk * 128], msg_dt, kind="ExternalInput")
    xt_in = nc.dram_tensor("xt", [P, npad], F32, kind="ExternalInput")
    eps_in = nc.dram_tensor("epsc", [P, 1], F32, kind="ExternalInput")
    w1t_in = nc.dram_tensor("w1t", [D, H], F32, kind="ExternalInput")
    w2ta_in = nc.dram_tensor("w2ta", [P, D], F32, kind="ExternalInput")
    w2tb_in = nc.dram_tensor("w2tb", [P, D], F32, kind="ExternalInput")
    g1_in = nc.dram_tensor("g1c", [P, 2], F32, kind="ExternalInput")
    b1_in = nc.dram_tensor("b1c", [P, 2], F32, kind="ExternalInput")
    g2_in = nc.dram_tensor("g2c", [P, 1], F32, kind="ExternalInput")
    b2_in = nc.dram_tensor("b2c", [P, 1], F32, kind="ExternalInput")
    out_t = nc.dram_tensor("outT", [P, npad], F32, kind="ExternalOutput")

    CW = 512                       # free-dim chunk for the MLP phases
    nch = (npad + CW - 1) // CW
    wlast = npad - CW * (nch - 1)
    # number of valid (non-pad) columns in the final chunk
    vlast = npc - CW * (nch - 1)
    assert 0 < vlast <= wlast

    with tile.TileContext(nc) as tc:
        with tc.tile_pool(name="persist", bufs=1) as pp:
            ht = pp.tile([P, npad], F32)          # h transposed, SBUF-resident
            w1t = pp.tile([P, H], F32)
            nc.sync.dma_start(w1t[:], w1t_in[:, :])
            w2ta = pp.tile([P, D], F32)
            nc.sync.dma_start(w2ta[:], w2ta_in[:, :])
            w2tb = pp.tile([P, D], F32)
            nc.sync.dma_start(w2tb[:], w2tb_in[:, :])
            g1c = pp.tile([P, 2], F32)
            nc.sync.dma_start(g1c[:], g1_in[:, :])
            b1c = pp.tile([P, 2], F32)
            nc.sync.dma_start(b1c[:], b1_in[:, :])
            g2c = pp.tile([P, 1], F32)
            nc.sync.dma_start(g2c[:], g2_in[:, :])
            b2c = pp.tile([P, 1], F32)
            nc.sync.dma_start(b2c[:], b2_in[:, :])
            epsc = pp.tile([P, 1], F32)
            nc.sync.dma_start(epsc[:], eps_in[:, :])
            scale = pp.tile([P, 1], F32)
            nc.vector.tensor_scalar_add(scale[:], epsc[:], 1.0)
            bneps = pp.tile([P, 1], F32)
            nc.vector.memset(bneps[:], BN_EPS)

            # ---------------- phase 1: message passing ----------------
            with tc.tile_pool(name="p1", bufs=1) as p1, \
                 tc.tile_pool(name="p1g", bufs=2) as p1g, \
                 tc.tile_pool(name="p1ps", bufs=4, space="PSUM") as p1ps:
                gxs = p1.tile([P, totk * 8], I16)
                nc.sync.dma_start(gxs[:], gx_in[:, :])
                lids = p1.tile([P, totk], F32)
                nc.sync.dma_start(lids[:], lid_in[:, :])
                iota = p1.tile([P, kmax * 128], F32)
                nc.gpsimd.iota(iota[:], pattern=[[0, kmax], [1, 128]], base=0,
                               channel_multiplier=0,
                               allow_small_or_imprecise_dtypes=True)

                for t in range(nt):
                    kt = int(k_t[t])
                    c0 = int(koff[t])
                    xg = p1g.tile([P, kt * 128], msg_dt, tag="xg",
                                  padded_shape=[P, kmax * 128])
                    for b in range(nb):
                        kbb = int(kb[t, b])
                        if kbb == 0:
                            continue
                        g0 = int(gc0[t, b]) - c0           # chunk offset in tile
                        ni = kbb * 128
                        nc.gpsimd.dma_gather(
                            out_ap=xg[:, g0 * 128:(g0 + kbb) * 128]
                                .rearrange("p (k d) -> p k d", d=128),
                            in_ap=x_in[b * BANK:min((b + 1) * BANK, n), :],
                            idxs_ap=gxs[:, (c0 + g0) * 8:(c0 + g0 + kbb) * 8],
                            num_idxs=ni,
                            num_idxs_reg=ni,
                            elem_size=128,
                        )
                    eat = p1g.tile([P, kt * 128], msg_dt, tag="ea",
                                   padded_shape=[P, kmax * 128])
                    nc.sync.dma_start(eat[:], ea_in[:, c0 * 128:(c0 + kt) * 128])
                    st_ = p1g.tile([P, kt * 128], msg_dt, tag="S",
                                   padded_shape=[P, kmax * 128])
                    nc.vector.tensor_tensor(
                        out=st_[:].rearrange("p (k d) -> p k d", d=128),
                        in0=lids[:, c0:c0 + kt].to_broadcast([P, kt, 128]),
                        in1=iota[:, :kt * 128].rearrange("p (k d) -> p k d", d=128),
                        op=mybir.AluOpType.is_equal,
                    )
                    ps = p1ps.tile([P, P], F32, space="PSUM", tag="ps")
                    if msg_dt == F32:
                        # r = xg + ea, then one matmul per chunk
                        nc.vector.tensor_tensor(out=xg[:], in0=xg[:], in1=eat[:],
                                                op=mybir.AluOpType.add)
                        for k in range(kt):
                            nc.tensor.matmul(ps[:],
                                             lhsT=xg[:, k * 128:(k + 1) * 128],
                                             rhs=st_[:, k * 128:(k + 1) * 128],
                                             start=(k == 0), stop=(k == kt - 1))
                    else:
                        # two bf16 matmuls per chunk (PSUM does the add)
                        for k in range(kt):
                            nc.tensor.matmul(ps[:],
                                             lhsT=xg[:, k * 128:(k + 1) * 128],
                                             rhs=st_[:, k * 128:(k + 1) * 128],
                                             start=(k == 0), stop=False)
                            nc.tensor.matmul(ps[:],
                                             lhsT=eat[:, k * 128:(k + 1) * 128],
                                             rhs=st_[:, k * 128:(k + 1) * 128],
                                             start=False, stop=(k == kt - 1))
                    # epilogue: hT[:, tile] = psum + (1+eps) * xT[:, tile]
                    xtt = p1g.tile([P, P], F32, tag="xt")
                    nc.sync.dma_start(xtt[:], xt_in[:, t * P:(t + 1) * P])
                    xts = p1g.tile([P, P], F32, tag="xts")
                    nc.vector.tensor_scalar_mul(xts[:], xtt[:], scale[:, :])
                    nc.vector.tensor_tensor(out=ht[:, t * P:(t + 1) * P],
                                            in0=ps[:], in1=xts[:],
                                            op=mybir.AluOpType.add)

            # ------- phase 2: z1 = W1 h (stats only, z1 recomputed later) -------
            with tc.tile_pool(name="z1p", bufs=1) as z1p:
                stats1 = z1p.tile([P, 4], F32)
                bn1 = z1p.tile([P, 8], F32)
                with tc.tile_pool(name="p2", bufs=1) as p2, \
                     tc.tile_pool(name="p2s", bufs=3) as p2s, \
                     tc.tile_pool(name="p2ps", bufs=4, space="PSUM") as p2ps:
                    redc = p2.tile([P, 4 * nch], F32)
                    for ci in range(nch):
                        s0 = ci * CW
                        w = CW if ci < nch - 1 else wlast
                        for hh in (0, 1):
                            psz = p2ps.tile([P, CW], F32, space="PSUM", tag="psz")
                            nc.tensor.matmul(psz[:, :w],
                                             lhsT=w1t[:, hh * 128:(hh + 1) * 128],
                                             rhs=ht[:, s0:s0 + w],
                                             start=True, stop=True)
                            nc.vector.reduce_sum(
                                redc[:, hh * nch + ci:hh * nch + ci + 1],
                                psz[:, :w], axis=mybir.AxisListType.X)
                            scr = p2s.tile([P, CW], F32, tag="scr")
                            nc.scalar.activation(
                                scr[:, :w], psz[:, :w],
                                mybir.ActivationFunctionType.Square,
                                accum_out=redc[:, (2 + hh) * nch + ci:
                                               (2 + hh) * nch + ci + 1])
                    for j in range(4):
                        nc.vector.reduce_sum(stats1[:, j:j + 1],
                                             redc[:, j * nch:(j + 1) * nch],
                                             axis=mybir.AxisListType.X)
                # all-reduce BN1 stats
                with tc.tile_pool(name="cc1", bufs=1, space="DRAM") as cc1p:
                    cc_in = cc1p.tile([P, 4], F32)
                    cc_out = cc1p.tile([P, 4], F32, addr_space="Shared")
                    nc.gpsimd.dma_start(cc_in[:], stats1[:])
                    nc.gpsimd.collective_compute(
                        "AllReduce", mybir.AluOpType.add,
                        replica_groups=[list(range(NCORES))],
                        ins=[cc_in[:]], outs=[cc_out[:]])
                    nc.gpsimd.dma_start(stats1[:], cc_out[:])
                # a1 = g1/std, c1 = b1 - mean*a1   (cols: [sum_a,sum_b,sq_a,sq_b])
                mean1, var1, a1, c1 = (bn1[:, 0:2], bn1[:, 2:4],
                                       bn1[:, 4:6], bn1[:, 6:8])
                nc.vector.tensor_scalar_mul(mean1, stats1[:, 0:2], 1.0 / n)
                nc.vector.tensor_scalar_mul(var1, stats1[:, 2:4], 1.0 / n)
                nc.vector.tensor_tensor(out=a1, in0=mean1, in1=mean1,
                                        op=mybir.AluOpType.mult)
                nc.vector.tensor_tensor(out=var1, in0=var1, in1=a1,
                                        op=mybir.AluOpType.subtract)
                nc.scalar.activation(var1, var1,
                                     mybir.ActivationFunctionType.Sqrt,
                                     bias=bneps[:, :])
                nc.vector.reciprocal(var1, var1)
                nc.vector.tensor_tensor(out=a1, in0=g1c[:, :], in1=var1,
                                        op=mybir.AluOpType.mult)
                nc.vector.tensor_tensor(out=c1, in0=mean1, in1=a1,
                                        op=mybir.AluOpType.mult)
                nc.vector.tensor_tensor(out=c1, in0=b1c[:, :], in1=c1,
                                        op=mybir.AluOpType.subtract)

                # --- phase 3: recompute z1, y1 = relu(bn1), z2 = W2 y1, stats ---
                with tc.tile_pool(name="z2p", bufs=1) as z2p:
                    z2 = z2p.tile([P, npad], F32)
                    stats2 = z2p.tile([P, 2], F32)
                    bn2 = z2p.tile([P, 4], F32)
                    with tc.tile_pool(name="p3", bufs=1) as p3, \
                         tc.tile_pool(name="p3s", bufs=3) as p3s, \
                         tc.tile_pool(name="p3ps", bufs=4, space="PSUM") as p3ps:
                        redc2 = p3.tile([P, 2 * nch], F32)
                        for ci in range(nch):
                            s0 = ci * CW
                            w = CW if ci < nch - 1 else wlast
                            v = CW if ci < nch - 1 else vlast
                            ps2 = p3ps.tile([P, CW], F32, space="PSUM", tag="ps2")
                            for hh in (0, 1):
                                psz2 = p3ps.tile([P, CW], F32, space="PSUM",
                                                 tag="psz2")
                                nc.tensor.matmul(psz2[:, :w],
                                                 lhsT=w1t[:, hh * 128:(hh + 1) * 128],
                                                 rhs=ht[:, s0:s0 + w],
                                                 start=True, stop=True)
                                y1 = p3s.tile([P, CW], F32, tag="y1")
                                nc.scalar.activation(
                                    y1[:, :w], psz2[:, :w],
                                    mybir.ActivationFunctionType.Relu,
                                    bias=c1[:, hh:hh + 1], scale=a1[:, hh:hh + 1])
                                nc.tensor.matmul(ps2[:, :w],
                                                 lhsT=(w2ta[:] if hh == 0 else w2tb[:]),
                                                 rhs=y1[:, :w],
                                                 start=(hh == 0), stop=(hh == 1))
                            nc.vector.tensor_copy(z2[:, s0:s0 + w], ps2[:, :w])
                            nc.vector.reduce_sum(redc2[:, ci:ci + 1],
                                                 z2[:, s0:s0 + v],
                                                 axis=mybir.AxisListType.X)
                            scr2 = p3s.tile([P, CW], F32, tag="scr2")
                            nc.scalar.activation(
                                scr2[:, :v], z2[:, s0:s0 + v],
                                mybir.ActivationFunctionType.Square,
                                accum_out=redc2[:, nch + ci:nch + ci + 1])
                        nc.vector.reduce_sum(stats2[:, 0:1], redc2[:, :nch],
                                             axis=mybir.AxisListType.X)
                        nc.vector.reduce_sum(stats2[:, 1:2], redc2[:, nch:],
                                             axis=mybir.AxisListType.X)
                    with tc.tile_pool(name="cc2", bufs=1, space="DRAM") as cc2p:
                        cc2_in = cc2p.tile([P, 2], F32)
                        cc2_out = cc2p.tile([P, 2], F32, addr_space="Shared")
                        nc.gpsimd.dma_start(cc2_in[:], stats2[:])
                        nc.gpsimd.collective_compute(
                            "AllReduce", mybir.AluOpType.add,
                            replica_groups=[list(range(NCORES))],
                            ins=[cc2_in[:]], outs=[cc2_out[:]])
                        nc.gpsimd.dma_start(stats2[:], cc2_out[:])
                    mean2, var2 = bn2[:, 0:1], bn2[:, 1:2]
                    a2, c2 = bn2[:, 2:3], bn2[:, 3:4]
                    nc.vector.tensor_scalar_mul(mean2, stats2[:, 0:1], 1.0 / n)
                    nc.vector.tensor_scalar_mul(var2, stats2[:, 1:2], 1.0 / n)
                    nc.vector.tensor_tensor(out=a2, in0=mean2, in1=mean2,
                                            op=mybir.AluOpType.mult)
                    nc.vector.tensor_tensor(out=var2, in0=var2, in1=a2,
                                            op=mybir.AluOpType.subtract)
                    nc.scalar.activation(var2, var2,
                                         mybir.ActivationFunctionType.Sqrt,
                                         bias=bneps[:, :])
                    nc.vector.reciprocal(var2, var2)
                    nc.vector.tensor_tensor(out=a2, in0=g2c[:, :], in1=var2,
                                            op=mybir.AluOpType.mult)
                    nc.vector.tensor_tensor(out=c2, in0=mean2, in1=a2,
                                            op=mybir.AluOpType.mult)
                    nc.vector.tensor_tensor(out=c2, in0=b2c[:, :], in1=c2,
                                            op=mybir.AluOpType.subtract)

                    # ------------- final: out = relu(a2*z2 + c2) -------------
                    with tc.tile_pool(name="p4s", bufs=3) as p4s:
                        for ci in range(nch):
                            s0 = ci * CW
                            w = CW if ci < nch - 1 else wlast
                            ot = p4s.tile([P, CW], F32, tag="ot")
                            nc.scalar.activation(
                                ot[:, :w], z2[:, s0:s0 + w],
                                mybir.ActivationFunctionType.Relu,
                                bias=c2[:, 0:1], scale=a2[:, 0:1])
                            nc.sync.dma_start(out_t[:, s0:s0 + w], ot[:, :w])
    nc.compile()
    return nc


# ----------------------------------------------------------------------------
# Entry point
# ----------------------------------------------------------------------------

_CACHE = {}

MSG_DTYPE = "float32"        # "float32" or "bfloat16" for the gather phase


def prepare(node_rep, edge_attr, degree, eps, W1, g1, b1, W2, g2, b2, src, dst):
    """Build (cached) device program + per-core input maps."""
    node_rep = np.asarray(node_rep, np.float32)
    edge_attr = np.asarray(edge_attr, np.float32)
    eps = np.asarray(eps, np.float32)
    W1 = np.asarray(W1, np.float32)
    W2 = np.asarray(W2, np.float32)
    g1 = np.asarray(g1, np.float32)
    b1 = np.asarray(b1, np.float32)
    g2 = np.asarray(g2, np.float32)
    b2 = np.asarray(b2, np.float32)
    src = np.asarray(src).astype(np.int64)
    dst = np.asarray(dst).astype(np.int64)

    n, d = node_rep.shape
    assert n % NCORES == 0 and d == 128
    msg_dt = F32 if MSG_DTYPE == "float32" else mybir.dt.bfloat16
    msg_np = np.float32 if MSG_DTYPE == "float32" else mybir.dt.np(mybir.dt.bfloat16)

    st = _build_structure(src, dst, n)
    npc, nt = st["npc"], st["nt"]
    npad = nt * P

    ckey = (n, edge_attr.shape[0], tuple(st["k_t"]), MSG_DTYPE)
    if ckey not in _CACHE:
        _CACHE.clear()
        _CACHE[ckey] = _build_nc(st, n, msg_dt)
    nc = _CACHE[ckey]

    x_msg = node_rep.astype(msg_np)
    epsc = np.broadcast_to(eps.reshape(1, 1), (P, 1)).astype(np.float32).copy()
    w1t = np.ascontiguousarray(W1.T)                     # [D, H]
    w2ta = np.ascontiguousarray(W2.T[:128])              # [H0, D]
    w2tb = np.ascontiguousarray(W2.T[128:])              # [H1, D]
    g1c = np.stack([g1[:128], g1[128:]], axis=1).astype(np.float32)
    b1c = np.stack([b1[:128], b1[128:]], axis=1).astype(np.float32)
    g2c = g2.reshape(P, 1).astype(np.float32)
    b2c = b2.reshape(P, 1).astype(np.float32)

    in_maps = []
    for c in range(NCORES):
        pc = _pack_core(st, c, node_rep, edge_attr, msg_np)
        in_maps.append({
            "x": x_msg, "gx": pc["gx"], "lid": pc["lid"], "eap": pc["eap"],
            "xt": pc["xt"], "epsc": epsc, "w1t": w1t, "w2ta": w2ta,
            "w2tb": w2tb, "g1c": g1c, "b1c": b1c, "g2c": g2c, "b2c": b2c,
        })

    return nc, in_maps, dict(npc=npc, nt=nt)


def kernel(**inputs):
    nc, in_maps, meta = prepare(**inputs)
    r = bass_utils.run_bass_kernel_spmd(nc, in_maps,
                                        core_ids=list(range(NCORES)))
    npc = meta["npc"]
    return np.concatenate(
        [r.results[c]["outT"][:, :npc].T for c in range(NCORES)], axis=0)


# revision 7
# speedup vs baseline: 49.1363x; 49.1363x over previous
"""GNN message-passing + MLP/BN kernel for 8 trn2 NeuronCores.

Math (algebraically identical to the reference; the degree terms cancel):
    h[v]  = (1+eps)*x[v] + sum_{edges e incident to v} (x[other(e)] + ea[e])
    z1    = h @ W1.T ; y1 = relu(bn1(z1))   (BatchNorm over the full batch)
    z2    = y1 @ W2.T; out = relu(bn2(z2))

Distribution: nodes are block-partitioned across 8 cores (12500 each).
The host builds, per core, an incidence list sorted by (node tile of 128,
bank of the gathered endpoint); edge features are packed per core in
incidence order (graph partitioning / halo duplication), node features are
replicated per core and gathered on-device with dma_gather. Scatter-add
into nodes is a one-hot matmul accumulated in PSUM. BatchNorm statistics
are all-reduced across the 8 cores on-device.
"""
import os
import sys
import numpy as np

for _p in ("/opt/trn_rl_repo", "/root/.axon_site/_ro/trn_rl_repo"):
    if os.path.isdir(_p) and _p not in sys.path:
        sys.path.insert(0, _p)

import concourse.bacc as bacc
import concourse.bass as bass
import concourse.mybir as mybir
import concourse.tile as tile
from concourse import bass_utils

NCORES = 8
BANK = 25000          # dma_gather int16 index range per bank
BN_EPS = 1e-5
P = 128

F32 = mybir.dt.float32
I16 = mybir.dt.int16


# ----------------------------------------------------------------------------
# Host-side graph structure
# ----------------------------------------------------------------------------

def _build_structure(src, dst, n):
    """Sorted incidence structure shared by all cores.

    Returns dict with per-(core,tile,bank) slot assignment of the 2E
    incidences (owner node, other endpoint, edge id)."""
    e = src.shape[0]
    npc = n // NCORES
    nt = (npc + P - 1) // P
    nb = (n + BANK - 1) // BANK

    owner = np.concatenate([src, dst])
    other = np.concatenate([dst, src])
    eid = np.concatenate([np.arange(e, dtype=np.int64)] * 2)

    core_k = owner // npc
    tile_k = (owner % npc) // P
    bank_k = other // BANK
    key = (core_k * nt + tile_k) * nb + bank_k
    order = np.argsort(key, kind="stable")
    owner_s = owner[order]
    other_s = other[order]
    eid_s = eid[order]
    key_s = key[order]

    counts = np.bincount(key_s, minlength=NCORES * nt * nb).reshape(NCORES, nt, nb)
    kb = -(-counts // P)            # ceil chunks per (core,tile,bank)
    kb = kb.max(axis=0)             # shared across cores [nt, nb]
    kb[:, 0] = np.maximum(kb[:, 0], 1)   # every tile gets >=1 chunk
    k_t = kb.sum(axis=1)            # chunks per tile [nt]
    koff = np.concatenate([[0], np.cumsum(k_t)])[:-1]          # chunk offset per tile
    kboff = np.concatenate([np.zeros((nt, 1), np.int64),
                            np.cumsum(kb, axis=1)], axis=1)[:, :-1]  # within tile
    totk = int(k_t.sum())

    # group chunk base for (t,b): gc0 = koff[t] + kboff[t,b]
    gc0 = koff[:, None] + kboff                                 # [nt, nb]

    # per-incidence destination slot: gc0[t,b]*128 + rank within (c,t,b) run
    run_bounds = np.concatenate([[0], np.cumsum(counts.reshape(-1))])
    rank = np.arange(owner.shape[0], dtype=np.int64) - run_bounds[key_s]
    t_of = (owner_s % npc) // P
    b_of = other_s // BANK
    dest = gc0[t_of, b_of] * P + rank                           # slot within core

    return dict(npc=npc, nt=nt, nb=nb, kb=kb, k_t=k_t, koff=koff, gc0=gc0,
                totk=totk, owner_s=owner_s, other_s=other_s, eid_s=eid_s,
                core_of=core_k[order], dest=dest)


def _pack_core(st, c, node_rep, edge_attr, msg_np):
    """Per-core input arrays."""
    npc, nt, totk = st["npc"], st["nt"], st["totk"]
    npad = nt * P
    sel = st["core_of"] == c
    owner = st["owner_s"][sel]
    other = st["other_s"][sel]
    eid = st["eid_s"][sel]
    dest = st["dest"][sel]

    base = c * npc
    # int16 gather indices, layout [16, totk*8] (idx j at (j%16, j//16)), x8 rows
    a16 = np.zeros((16, totk * 8), np.int16)
    a16[dest % 16, dest // 16] = (other % BANK).astype(np.int16)
    gx = np.tile(a16, (8, 1))

    lid = np.full((P, totk), -1.0, np.float32)
    lid[dest % P, dest // P] = (owner - base) % P

    eidx = np.zeros((P, totk), np.int64)
    eidx[dest % P, dest // P] = eid
    eap = edge_attr[eidx.reshape(-1)].astype(msg_np).reshape(P, totk * 128)

    xt = np.zeros((P, npad), np.float32)
    xt[:, :npc] = node_rep[base:base + npc].T
    return dict(gx=gx, lid=lid, eap=eap, xt=xt)


# ----------------------------------------------------------------------------
# Device program
# ----------------------------------------------------------------------------

def _build_nc(st, n, msg_dt, repeat=1):
    npc, nt, nb, totk = st["npc"], st["nt"], st["nb"], st["totk"]
    kb, k_t, koff, gc0 = st["kb"], st["k_t"], st["koff"], st["gc0"]
    npad = nt * P
    kmax = int(k_t.max())
    H = 256
    D = 128

    nc = bacc.Bacc("TRN2", target_bir_lowering=False, debug=False,
                   num_devices=NCORES)
    x_in = nc.dram_tensor("x", [n, D], msg_dt, kind="ExternalInput")
    gx_in = nc.dram_tensor("gx", [P, totk * 8], I16, kind="ExternalInput")
    lid_in = nc.dram_tensor("lid", [P, totk], F32, kind="ExternalInput")
    ea_in = nc.dram_tensor("eap", [P, totk * 128], msg_dt, kind="ExternalInput")
    xt_in = nc.dram_tensor("xt", [P, npad], F32, kind="ExternalInput")
    eps_in = nc.dram_tensor("epsc", [P, 1], F32, kind="ExternalInput")
    w1t_in = nc.dram_tensor("w1t", [D, H], F32, kind="ExternalInput")
    w2ta_in = nc.dram_tensor("w2ta", [P, D], F32, kind="ExternalInput")
    w2tb_in = nc.dram_tensor("w2tb", [P, D], F32, kind="ExternalInput")
    g1_in = nc.dram_tensor("g1c", [P, 2], F32, kind="ExternalInput")
    b1_in = nc.dram_tensor("b1c", [P, 2], F32, kind="ExternalInput")
    g2_in = nc.dram_tensor("g2c", [P, 1], F32, kind="ExternalInput")
    b2_in = nc.dram_tensor("b2c", [P, 1], F32, kind="ExternalInput")
    out_t = nc.dram_tensor("outT", [P, npad], F32, kind="ExternalOutput")

    CW = 512                       # free-dim chunk for the MLP phases
    nch = (npad + CW - 1) // CW
    wlast = npad - CW * (nch - 1)
    # number of valid (non-pad) columns in the final chunk
    vlast = npc - CW * (nch - 1)
    assert 0 < vlast <= wlast

    with tile.TileContext(nc) as tc:
      for _rep in range(repeat):
        with tc.tile_pool(name="persist", bufs=1) as pp:
            ht = pp.tile([P, npad], F32)          # h transposed, SBUF-resident
            w1t = pp.tile([P, H], F32)
            nc.sync.dma_start(w1t[:], w1t_in[:, :])
            w2ta = pp.tile([P, D], F32)
            nc.sync.dma_start(w2ta[:], w2ta_in[:, :])
            w2tb = pp.tile([P, D], F32)
            nc.sync.dma_start(w2tb[:], w2tb_in[:, :])
            g1c = pp.tile([P, 2], F32)
            nc.sync.dma_start(g1c[:], g1_in[:, :])
            b1c = pp.tile([P, 2], F32)
            nc.sync.dma_start(b1c[:], b1_in[:, :])
            g2c = pp.tile([P, 1], F32)
            nc.sync.dma_start(g2c[:], g2_in[:, :])
            b2c = pp.tile([P, 1], F32)
            nc.sync.dma_start(b2c[:], b2_in[:, :])
            epsc = pp.tile([P, 1], F32)
            nc.sync.dma_start(epsc[:], eps_in[:, :])
            scale = pp.tile([P, 1], F32)
            nc.vector.tensor_scalar_add(scale[:], epsc[:], 1.0)
            bneps = pp.tile([P, 1], F32)
            nc.vector.memset(bneps[:], BN_EPS)

            # ---------------- phase 1: message passing ----------------
            with tc.tile_pool(name="p1", bufs=1) as p1, \
                 tc.tile_pool(name="p1g", bufs=2) as p1g, \
                 tc.tile_pool(name="p1ps", bufs=4, space="PSUM") as p1ps:
                gxs = p1.tile([P, totk * 8], I16)
                nc.sync.dma_start(gxs[:], gx_in[:, :])
                lids = p1.tile([P, totk], F32)
                nc.sync.dma_start(lids[:], lid_in[:, :])
                iota = p1.tile([P, kmax * 128], F32)
                nc.gpsimd.iota(iota[:], pattern=[[0, kmax], [1, 128]], base=0,
                               channel_multiplier=0,
                               allow_small_or_imprecise_dtypes=True)

                for t in range(nt):
                    kt = int(k_t[t])
                    c0 = int(koff[t])
                    xg = p1g.tile([P, kt * 128], msg_dt, tag="xg",
                                  padded_shape=[P, kmax * 128])
                    for b in range(nb):
                        kbb = int(kb[t, b])
                        if kbb == 0:
                            continue
                        g0 = int(gc0[t, b]) - c0           # chunk offset in tile
                        ni = kbb * 128
                        nc.gpsimd.dma_gather(
                            out_ap=xg[:, g0 * 128:(g0 + kbb) * 128]
                                .rearrange("p (k d) -> p k d", d=128),
                            in_ap=x_in[b * BANK:min((b + 1) * BANK, n), :],
                            idxs_ap=gxs[:, (c0 + g0) * 8:(c0 + g0 + kbb) * 8],
                            num_idxs=ni,
                            num_idxs_reg=ni,
                            elem_size=128,
                        )
                    eat = p1g.tile([P, kt * 128], msg_dt, tag="ea",
                                   padded_shape=[P, kmax * 128])
                    nc.sync.dma_start(eat[:], ea_in[:, c0 * 128:(c0 + kt) * 128])
                    st_ = p1g.tile([P, kt * 128], msg_dt, tag="S",
                                   padded_shape=[P, kmax * 128])
                    nc.vector.tensor_tensor(
                        out=st_[:].rearrange("p (k d) -> p k d", d=128),
                        in0=lids[:, c0:c0 + kt].to_broadcast([P, kt, 128]),
                        in1=iota[:, :kt * 128].rearrange("p (k d) -> p k d", d=128),
                        op=mybir.AluOpType.is_equal,
                    )
                    ps = p1ps.tile([P, P], F32, space="PSUM", tag="ps")
                    if msg_dt == F32:
                        # r = xg + ea, then one matmul per chunk
                        nc.vector.tensor_tensor(out=xg[:], in0=xg[:], in1=eat[:],
                                                op=mybir.AluOpType.add)
                        for k in range(kt):
                            nc.tensor.matmul(ps[:],
                                             lhsT=xg[:, k * 128:(k + 1) * 128],
                                             rhs=st_[:, k * 128:(k + 1) * 128],
                                             start=(k == 0), stop=(k == kt - 1))
                    else:
                        # two bf16 matmuls per chunk (PSUM does the add)
                        for k in range(kt):
                            nc.tensor.matmul(ps[:],
                                             lhsT=xg[:, k * 128:(k + 1) * 128],
                                             rhs=st_[:, k * 128:(k + 1) * 128],
                                             start=(k == 0), stop=False)
                            nc.tensor.matmul(ps[:],
                                             lhsT=eat[:, k * 128:(k + 1) * 128],
                                             rhs=st_[:, k * 128:(k + 1) * 128],
                                             start=False, stop=(k == kt - 1))
                    # epilogue: hT[:, tile] = psum + (1+eps) * xT[:, tile]
                    xtt = p1g.tile([P, P], F32, tag="xt")
                    nc.sync.dma_start(xtt[:], xt_in[:, t * P:(t + 1) * P])
                    xts = p1g.tile([P, P], F32, tag="xts")
                    nc.vector.tensor_scalar_mul(xts[:], xtt[:], scale[:, :])
                    nc.vector.tensor_tensor(out=ht[:, t * P:(t + 1) * P],
                                            in0=ps[:], in1=xts[:],
                                            op=mybir.AluOpType.add)

            # ------- phase 2: z1 = W1 h (stats only, z1 recomputed later) -------
            with tc.tile_pool(name="z1p", bufs=1) as z1p:
                stats1 = z1p.tile([P, 4], F32)
                bn1 = z1p.tile([P, 8], F32)
                with tc.tile_pool(name="p2", bufs=1) as p2, \
                     tc.tile_pool(name="p2s", bufs=3) as p2s, \
                     tc.tile_pool(name="p2ps", bufs=4, space="PSUM") as p2ps:
                    redc = p2.tile([P, 4 * nch], F32)
                    for ci in range(nch):
                        s0 = ci * CW
                        w = CW if ci < nch - 1 else wlast
                        for hh in (0, 1):
                            psz = p2ps.tile([P, CW], F32, space="PSUM", tag="psz")
                            nc.tensor.matmul(psz[:, :w],
                                             lhsT=w1t[:, hh * 128:(hh + 1) * 128],
                                             rhs=ht[:, s0:s0 + w],
                                             start=True, stop=True)
                            nc.vector.reduce_sum(
                                redc[:, hh * nch + ci:hh * nch + ci + 1],
                                psz[:, :w], axis=mybir.AxisListType.X)
                            scr = p2s.tile([P, CW], F32, tag="scr")
                            nc.scalar.activation(
                                scr[:, :w], psz[:, :w],
                                mybir.ActivationFunctionType.Square,
                                accum_out=redc[:, (2 + hh) * nch + ci:
                                               (2 + hh) * nch + ci + 1])
                    for j in range(4):
                        nc.vector.reduce_sum(stats1[:, j:j + 1],
                                             redc[:, j * nch:(j + 1) * nch],
                                             axis=mybir.AxisListType.X)
                # all-reduce BN1 stats
                with tc.tile_pool(name="cc1", bufs=1, space="DRAM") as cc1p:
                    cc_in = cc1p.tile([P, 4], F32)
                    cc_out = cc1p.tile([P, 4], F32, addr_space="Shared")
                    nc.gpsimd.dma_start(cc_in[:], stats1[:])
                    nc.gpsimd.collective_compute(
                        "AllReduce", mybir.AluOpType.add,
                        replica_groups=[list(range(NCORES))],
                        ins=[cc_in[:]], outs=[cc_out[:]])
                    nc.gpsimd.dma_start(stats1[:], cc_out[:])
                # a1 = g1/std, c1 = b1 - mean*a1   (cols: [sum_a,sum_b,sq_a,sq_b])
                mean1, var1, a1, c1 = (bn1[:, 0:2], bn1[:, 2:4],
                                       bn1[:, 4:6], bn1[:, 6:8])
                nc.vector.tensor_scalar_mul(mean1, stats1[:, 0:2], 1.0 / n)
                nc.vector.tensor_scalar_mul(var1, stats1[:, 2:4], 1.0 / n)
                nc.vector.tensor_tensor(out=a1, in0=mean1, in1=mean1,
                                        op=mybir.AluOpType.mult)
                nc.vector.tensor_tensor(out=var1, in0=var1, in1=a1,
                                        op=mybir.AluOpType.subtract)
                nc.scalar.activation(var1, var1,
                                     mybir.ActivationFunctionType.Sqrt,
                                     bias=bneps[:, :])
                nc.vector.reciprocal(var1, var1)
                nc.vector.tensor_tensor(out=a1, in0=g1c[:, :], in1=var1,
                                        op=mybir.AluOpType.mult)
                nc.vector.tensor_tensor(out=c1, in0=mean1, in1=a1,
                                        op=mybir.AluOpType.mult)
                nc.vector.tensor_tensor(out=c1, in0=b1c[:, :], in1=c1,
                                        op=mybir.AluOpType.subtract)

                # --- phase 3: recompute z1, y1 = relu(bn1), z2 = W2 y1, stats ---
                with tc.tile_pool(name="z2p", bufs=1) as z2p:
                    z2 = z2p.tile([P, npad], F32)
                    stats2 = z2p.tile([P, 2], F32)
                    bn2 = z2p.tile([P, 4], F32)
                    with tc.tile_pool(name="p3", bufs=1) as p3, \
                         tc.tile_pool(name="p3s", bufs=3) as p3s, \
                         tc.tile_pool(name="p3ps", bufs=4, space="PSUM") as p3ps:
                        redc2 = p3.tile([P, 2 * nch], F32)
                        for ci in range(nch):
                            s0 = ci * CW
                            w = CW if ci < nch - 1 else wlast
                            v = CW if ci < nch - 1 else vlast
                            ps2 = p3ps.tile([P, CW], F32, space="PSUM", tag="ps2")
                            for hh in (0, 1):
                                psz2 = p3ps.tile([P, CW], F32, space="PSUM",
                                                 tag="psz2")
                                nc.tensor.matmul(psz2[:, :w],
                                                 lhsT=w1t[:, hh * 128:(hh + 1) * 128],
                                                 rhs=ht[:, s0:s0 + w],
                                                 start=True, stop=True)
                                y1 = p3s.tile([P, CW], F32, tag="y1")
                                nc.scalar.activation(
                                    y1[:, :w], psz2[:, :w],
                                    mybir.ActivationFunctionType.Relu,
                                    bias=c1[:, hh:hh + 1], scale=a1[:, hh:hh + 1])
                                nc.tensor.matmul(ps2[:, :w],
                                                 lhsT=(w2ta[:] if hh == 0 else w2tb[:]),
                                                 rhs=y1[:, :w],
                                                 start=(hh == 0), stop=(hh == 1))
                            nc.vector.tensor_copy(z2[:, s0:s0 + w], ps2[:, :w])
                            nc.vector.reduce_sum(redc2[:, ci:ci + 1],
                                                 z2[:, s0:s0 + v],
                                                 axis=mybir.AxisListType.X)
                            scr2 = p3s.tile([P, CW], F32, tag="scr2")
                            nc.scalar.activation(
                                scr2[:, :v], z2[:, s0:s0 + v],
                                mybir.ActivationFunctionType.Square,
                                accum_out=redc2[:, nch + ci:nch + ci + 1])
                        nc.vector.reduce_sum(stats2[:, 0:1], redc2[:, :nch],
                                             axis=mybir.AxisListType.X)
                        nc.vector.reduce_sum(stats2[:, 1:2], redc2[:, nch:],
                                             axis=mybir.AxisListType.X)
                    with tc.tile_pool(name="cc2", bufs=1, space="DRAM") as cc2p:
                        cc2_in = cc2p.tile([P, 2], F32)
                        cc2_out = cc2p.tile([P, 2], F32, addr_space="Shared")
                        nc.gpsimd.dma_start(cc2_in[:], stats2[:])
                        nc.gpsimd.collective_compute(
                            "AllReduce", mybir.AluOpType.add,
                            replica_groups=[list(range(NCORES))],
                            ins=[cc2_in[:]], outs=[cc2_out[:]])
                        nc.gpsimd.dma_start(stats2[:], cc2_out[:])
                    mean2, var2 = bn2[:, 0:1], bn2[:, 1:2]
                    a2, c2 = bn2[:, 2:3], bn2[:, 3:4]
                    nc.vector.tensor_scalar_mul(mean2, stats2[:, 0:1], 1.0 / n)
                    nc.vector.tensor_scalar_mul(var2, stats2[:, 1:2], 1.0 / n)
                    nc.vector.tensor_tensor(out=a2, in0=mean2, in1=mean2,
                                            op=mybir.AluOpType.mult)
                    nc.vector.tensor_tensor(out=var2, in0=var2, in1=a2,
                                            op=mybir.AluOpType.subtract)
                    nc.scalar.activation(var2, var2,
                                         mybir.ActivationFunctionType.Sqrt,
                                         bias=bneps[:, :])
                    nc.vector.reciprocal(var2, var2)
                    nc.vector.tensor_tensor(out=a2, in0=g2c[:, :], in1=var2,
                                            op=mybir.AluOpType.mult)
                    nc.vector.tensor_tensor(out=c2, in0=mean2, in1=a2,
                                            op=mybir.AluOpType.mult)
                    nc.vector.tensor_tensor(out=c2, in0=b2c[:, :], in1=c2,
                                            op=mybir.AluOpType.subtract)

                    # ------------- final: out = relu(a2*z2 + c2) -------------
                    with tc.tile_pool(name="p4s", bufs=3) as p4s:
                        for ci in range(nch):
                            s0 = ci * CW
                            w = CW if ci < nch - 1 else wlast
                            ot = p4s.tile([P, CW], F32, tag="ot")
                            nc.scalar.activation(
                                ot[:, :w], z2[:, s0:s0 + w],
                                mybir.ActivationFunctionType.Relu,
                                bias=c2[:, 0:1], scale=a2[:, 0:1])
                            nc.sync.dma_start(out_t[:, s0:s0 + w], ot[:, :w])
    nc.compile()
    return nc


# ----------------------------------------------------------------------------
# Entry point
# ----------------------------------------------------------------------------

_CACHE = {}

MSG_DTYPE = "float32"        # "float32" or "bfloat16" for the gather phase
REPEAT = 1                   # body repetitions (timing experiments only)


def prepare(node_rep, edge_attr, degree, eps, W1, g1, b1, W2, g2, b2, src, dst):
    """Build (cached) device program + per-core input maps."""
    node_rep = np.asarray(node_rep, np.float32)
    edge_attr = np.asarray(edge_attr, np.float32)
    eps = np.asarray(eps, np.float32)
    W1 = np.asarray(W1, np.float32)
    W2 = np.asarray(W2, np.float32)
    g1 = np.asarray(g1, np.float32)
    b1 = np.asarray(b1, np.float32)
    g2 = np.asarray(g2, np.float32)
    b2 = np.asarray(b2, np.float32)
    src = np.asarray(src).astype(np.int64)
    dst = np.asarray(dst).astype(np.int64)

    n, d = node_rep.shape
    assert n % NCORES == 0 and d == 128
    msg_dt = F32 if MSG_DTYPE == "float32" else mybir.dt.bfloat16
    msg_np = np.float32 if MSG_DTYPE == "float32" else mybir.dt.np(mybir.dt.bfloat16)

    st = _build_structure(src, dst, n)
    npc, nt = st["npc"], st["nt"]
    npad = nt * P

    ckey = (n, edge_attr.shape[0], tuple(st["k_t"]), MSG_DTYPE, REPEAT)
    if ckey not in _CACHE:
        _CACHE.clear()
        _CACHE[ckey] = _build_nc(st, n, msg_dt, repeat=REPEAT)
    nc = _CACHE[ckey]

    x_msg = node_rep.astype(msg_np)
    epsc = np.broadcast_to(eps.reshape(1, 1), (P, 1)).astype(np.float32).copy()
    w1t = np.ascontiguousarray(W1.T)                     # [D, H]
    w2ta = np.ascontiguousarray(W2.T[:128])              # [H0, D]
    w2tb = np.ascontiguousarray(W2.T[128:])              # [H1, D]
    g1c = np.stack([g1[:128], g1[128:]], axis=1).astype(np.float32)
    b1c = np.stack([b1[:128], b1[128:]], axis=1).astype(np.float32)
    g2c = g2.reshape(P, 1).astype(np.float32)
    b2c = b2.reshape(P, 1).astype(np.float32)

    in_maps = []
    for c in range(NCORES):
        pc = _pack_core(st, c, node_rep, edge_attr, msg_np)
        in_maps.append({
            "x": x_msg, "gx": pc["gx"], "lid": pc["lid"], "eap": pc["eap"],
            "xt": pc["xt"], "epsc": epsc, "w1t": w1t, "w2ta": w2ta,
            "w2tb": w2tb, "g1c": g1c, "b1c": b1c, "g2c": g2c, "b2c": b2c,
        })

    return nc, in_maps, dict(npc=npc, nt=nt)


def kernel(**inputs):
    nc, in_maps, meta = prepare(**inputs)
    r = bass_utils.run_bass_kernel_spmd(nc, in_maps,
                                        core_ids=list(range(NCORES)))
    npc = meta["npc"]
    return np.concatenate(
        [r.results[c]["outT"][:, :npc].T for c in range(NCORES)], axis=0)
